# revision 62
# baseline (speedup 1.0000x reference)
"""CrossAttention (B=1, S=4096, H=8, DH=40) on 8 Trainium2 NeuronCores.

Sharding: tensor-parallel over the 8 heads — core h computes head h's full
attention plus its partial output projection; the host sums the 8 partials
and adds the bias.

v12 dataflow (uniform lag-1):
  warm-up   dense junk matmuls first: the PE clock release (1.2 -> 2.4 GHz)
            follows ~10us of gap-free activity, so warming starts before
            any real work; a dummy ACT preloads the exp table meanwhile
  preamble  prepacked weight images (single DMAs); xT in 1024-col slices
            over two issuing queues; qkA projections drain from idle ps_st
            banks (low chunk on DVE, high on ScalarE); qkB derived from
            qkA by two SBUF-SBUF DMAs (same data, partition halves
            swapped); v-projection and v' PE-transposes also live here
  ST        [128j, 512i] pairs on disjoint PE row groups -> exp on ScalarE
            [128, 1024] (the wall: ~16.7M exps at 1 elem/cycle/lane)
  AV        lag-1 everywhere: av(c) consumes chunk c's exp tiles one slot
            behind; pairs of chunks share one PSUM bank (even chunk at PE
            cols 0-40, odd at 64-104 via tile_position), pair banks
            alternate between ps_av and ps_warm
  norm      no broadcast, no row reciprocal: y_i = (o_i @ Wo) / r_i -- the
            r row is PE-transposed into a per-partition column, 1/r is a
            cheap [128,4] DVE reciprocal, and the divide folds into the
            y-tile PSUM drain as a per-partition tensor_scalar
  post      each chunk's half-pair post-processing (oT copy, r column,
            4 y tiles) runs early in the NEXT chunk; the tail handles only
            chunk 7's half
"""

import os

import ml_dtypes
import numpy as np

import concourse.bass as bass
import concourse.mybir as mybir
from concourse import bass_utils, masks
from concourse.tile import TileContext

S = 4096
D = 320
H = 8
DH = 40
N_CORES = 8
CHUNK = 512               # i-chunk width (one fp32 PSUM bank)
VW = 41                   # v' stationary width: 40 v cols, ones col 40
GJ = 2                    # j-tiles per exp group (2 PSUM banks)
SCALE = float(DH) ** -0.5
QKW = 104
BURST = int(os.environ.get('BASS_BURST', '16'))  # warm-up burst length

F32 = mybir.dt.float32
BF16 = mybir.dt.bfloat16
EXP = mybir.ActivationFunctionType.Exp
MULT = mybir.AluOpType.mult

KCH = (128, 128, 64)      # K chunks of D=320

_COMPILED = {}


def _split_sync_waits(nc, max_waits=1):
    """This walrus build rejects instructions with more than one sync wait.
    Spill the excess onto same-engine nops placed just before the
    instruction (engine streams execute in program order, so all waits are
    satisfied before the instruction issues)."""
    for f in nc.m.functions:
        for bb in f.blocks:
            out = []
            changed = False
            for inst in bb.instructions:
                si = inst.sync_info
                if si is not None and si.on_wait and len(si.on_wait) > max_waits:
                    waits = list(si.on_wait)
                    for i in range(max_waits, len(waits), max_waits):
                        nop = mybir.InstNoOp(
                            name=nc.get_next_instruction_name(),
                            engine=inst.engine,
                            bass_nofuse=True,
                            sync_info=mybir.SyncInfo(
                                on_wait=waits[i:i + max_waits], on_update=[]),
                        )
                        out.append(nop)
                    inst.sync_info = mybir.SyncInfo(
                        on_wait=waits[:max_waits],
                        on_update=list(si.on_update or []))
                    changed = True
                out.append(inst)
            if changed:
                bb.instructions = out


def _build(s=None, split=True):
    s = s or S
    n_chunks = s // CHUNK
    jt = s // 128
    gpc = jt // GJ            # exp groups per chunk
    tpc = CHUNK // 128        # s-tiles per chunk
    nc = bass.Bass('TRN2', target_bir_lowering=False, debug=False)

    xT_d = nc.dram_tensor('xT', [D, s], BF16, kind='ExternalInput').ap()
    wA_d = nc.dram_tensor('wA', [128, 3 * QKW], BF16, kind='ExternalInput').ap()
    wv_d = nc.dram_tensor('wv', [128, 3 * VW], BF16, kind='ExternalInput').ap()
    woT_d = nc.dram_tensor('woT', [64 + VW, D], BF16, kind='ExternalInput').ap()
    out_d = nc.dram_tensor('out', [s, D], F32, kind='ExternalOutput').ap()

    with TileContext(nc) as tc:
        with tc.tile_pool(name='const', bufs=1) as cpool, \
             tc.tile_pool(name='big', bufs=1) as big, \
             tc.tile_pool(name='pt', bufs=18) as ptp, \
             tc.tile_pool(name='work', bufs=3) as wkp, \
             tc.tile_pool(name='ps_st', bufs=2, space='PSUM') as ps_st, \
             tc.tile_pool(name='ps_small', bufs=2, space='PSUM') as ps_small, \
             tc.tile_pool(name='ps_av', bufs=1, space='PSUM') as ps_av, \
             tc.tile_pool(name='ps_warm', bufs=1, space='PSUM') as ps_warm:

            # ---- warm-up burst (first thing emitted) ----
            jsrc = cpool.tile([128, CHUNK], BF16, tag='jsrc')
            nc.vector.memset(jsrc[:, :], 0.25)
            for _ in range(BURST):
                wj = ps_warm.tile([128, CHUNK], F32, tag='av', name='wj')
                nc.tensor.matmul(wj[:, :], jsrc[:, 0:128], jsrc[:, :],
                                 start=True, stop=True)
            # dummy ACT: exp table-set load (~2.7us) during the DMA wait
            sc = wkp.tile([1, 16], F32, tag='sc')
            nc.scalar.activation(sc[0:1, :], jsrc[0:1, 0:16], EXP)

            # ---- constants & inputs ----
            ident = cpool.tile([128, 128], F32, tag='ident')
            masks.make_identity(nc, ident[:, :])

            wA = cpool.tile([128, 3 * QKW], BF16, tag='wA')
            wv_sb = cpool.tile([128, 3 * VW], BF16, tag='wv')
            woT2 = cpool.tile([64 + VW, D], BF16, tag='woT')
            nc.sync.dma_start(wA[:, :], wA_d)
            nc.scalar.dma_start(wv_sb[:, :], wv_d)
            nc.gpsimd.dma_start(woT2[:, :], woT_d)

            # xT in 1024-col slices; scalar queue stays clear of slice DMAs
            # so the table load + qkA drains there are never queued behind
            xt0 = big.tile([128, s], BF16, tag='xt0')
            xt1 = big.tile([128, s], BF16, tag='xt1')
            xt2 = big.tile([64, s], BF16, tag='xt2')
            xts = (xt0, xt1, xt2)
            for sl in range(s // 1024):
                ss = slice(sl * 1024, (sl + 1) * 1024)
                nc.sync.dma_start(xt0[:, ss], xT_d[0:128, ss])
                nc.sync.dma_start(xt1[:, ss], xT_d[128:256, ss])
                nc.gpsimd.dma_start(xt2[:, ss], xT_d[256:320, ss])

            qkA = big.tile([QKW, s], BF16, tag='qkA')
            qkB = big.tile([QKW, s], BF16, tag='qkB')
            vT = big.tile([VW, s], F32, tag='vT')
            vsb = big.tile([128, jt * VW], BF16, tag='vsb')
            oT_lo = big.tile([VW, s], BF16, tag='oT_lo')
            oT_hi = big.tile([64 + VW, s], BF16, tag='oT_hi')


            # ---- helpers ----
            def proj(dst, w_sb, c, ww):
                ps = ps_small.tile([QKW, CHUNK], F32, tag='small')
                for ci, kk in enumerate(KCH):
                    nc.tensor.matmul(
                        ps[0:ww, :],
                        w_sb[0:kk, ci * ww:(ci + 1) * ww],
                        xts[ci][0:kk, c * CHUNK:(c + 1) * CHUNK],
                        start=(ci == 0), stop=(ci == 2))
                nc.vector.tensor_copy(dst[:, c * CHUNK:(c + 1) * CHUNK],
                                      ps[0:ww, :])

            def proj_qk2(c2):
                # qkA projections for chunks 2*c2, 2*c2+1 into one 2-bank
                # ps_st tile (idle until the ST stream starts); 4-chunk WAR
                # distance keeps the PE clear of the drain round-trip
                ps = ps_st.tile([128, GJ * CHUNK], F32, tag='st2')
                for h in range(2):
                    c = 2 * c2 + h
                    cs = slice(c * CHUNK, (c + 1) * CHUNK)
                    for ci, kk in enumerate(KCH):
                        nc.tensor.matmul(
                            ps[0:QKW, h * CHUNK:(h + 1) * CHUNK],
                            wA[0:kk, ci * QKW:(ci + 1) * QKW],
                            xts[ci][0:kk, cs],
                            start=(ci == 0), stop=(ci == 2))
                c0s = slice(2 * c2 * CHUNK, (2 * c2 + 1) * CHUNK)
                c1s = slice((2 * c2 + 1) * CHUNK, (2 * c2 + 2) * CHUNK)
                nc.vector.tensor_copy(qkA[:, c0s], ps[0:QKW, 0:CHUNK])
                nc.scalar.copy(qkA[:, c1s], ps[0:QKW, CHUNK:2 * CHUNK])

            def transpose_v(j):
                tp = ps_small.tile([128, VW], F32, tag='small')
                nc.tensor.transpose(tp[:, 0:VW], vT[:, j * 128:(j + 1) * 128],
                                    ident[0:VW, 0:VW])
                nc.vector.tensor_copy(vsb[:, j * VW:(j + 1) * VW],
                                      tp[:, 0:VW])

            def warm(n=1):
                # junk keep-warm matmuls for thin chunk-0 slots
                for _ in range(n):
                    wj = ps_warm.tile([128, CHUNK], F32, tag='av',
                                      name='wj')
                    nc.tensor.matmul(wj[0:QKW, 0:384], wA[:, 0:QKW],
                                     xt0[:, 0:384], start=True, stop=True)

            av_tiles = {}

            def av_mm(c_src, pt, g):
                # two j-tile matmuls of group g, consuming chunk c_src's exp
                # tiles: even chunk at PE cols 0-40, odd at 64-104. Pair
                # banks alternate ps_av / ps_warm so a new pair never WARs
                # on the previous pair's drain copy.
                lo = (c_src % 2 == 1)
                pair = c_src // 2
                for k in range(GJ):
                    j = GJ * g + k
                    if j == 0 and c_src % 2 == 0:
                        pool = ps_av if pair % 2 == 0 else ps_warm
                        av_tiles[pair] = pool.tile(
                            [128, CHUNK], F32, tag='av', name='av')
                    av = av_tiles[pair]
                    if lo:
                        nc.tensor.matmul(
                            av[0:VW, :], vsb[:, j * VW:(j + 1) * VW],
                            pt[:, k * CHUNK:(k + 1) * CHUNK],
                            start=(j == 0), stop=(j == jt - 1))
                    else:
                        nc.tensor.matmul(
                            av[64:64 + VW, :], vsb[:, j * VW:(j + 1) * VW],
                            pt[:, k * CHUNK:(k + 1) * CHUNK],
                            start=(j == 0), stop=(j == jt - 1),
                            tile_position=(0, 64))

            half_state = {}

            def norm_a(c, tail=False):
                # drain this chunk's 41 AV rows (r row included at
                # partition 40 or 104 -- transposed in place by norm_c, no
                # partition-0 DMA hop needed)
                av = av_tiles[c // 2]
                row0 = 0 if c % 2 == 1 else 64
                m = wkp.tile([105, CHUNK], F32, tag='m')
                nc.vector.tensor_copy(m[row0:row0 + VW, :],
                                      av[row0:row0 + VW, :])
                half_state[c] = [m, None, None]

            def norm_b(c):
                # unnormalized oT image (bf16 copy; SBUF 2x mode)
                m = half_state[c][0]
                cs = slice(c * CHUNK, (c + 1) * CHUNK)
                if c % 2 == 1:
                    nc.vector.tensor_copy(oT_lo[:, cs], m[0:VW, :])
                else:
                    nc.vector.tensor_copy(oT_hi[64:64 + VW, cs],
                                          m[64:64 + VW, :])

            def norm_c(c):
                # PE-transpose the r row (read in place at partition 40 or
                # 104 -- SBUF partition k feeds PE array row k, so the
                # matching identity diagonal element makes the transpose
                # land in rr[:, t]) and take the cheap [128,4] reciprocal
                m = half_state[c][0]
                rr = ps_small.tile([128, 8], F32, tag='small')
                if c % 2 == 1:
                    src_r, rp = m, 32
                else:
                    src_r = wkp.tile([1, CHUNK], F32, tag='r')
                    nc.gpsimd.dma_start(src_r[0:1, :], m[96:97, :])
                    rp = 0
                for t in range(tpc):
                    nc.tensor.transpose(
                        rr[:, t:t + 1],
                        src_r[rp:rp + 1, t * 128:(t + 1) * 128],
                        ident[rp:rp + 1, rp:rp + 1])
                rcol = wkp.tile([128, 8], F32, tag='rcol')
                nc.vector.tensor_copy(rcol[:, 0:tpc], rr[:, 0:tpc])
                rinv = wkp.tile([128, 8], F32, tag='rinv')
                nc.vector.reciprocal(rinv[:, 0:tpc], rcol[:, 0:tpc])
                half_state[c][2] = rinv

            def outproj_tile(c, t, tail=False):
                # y tile = oT.T @ Wo_h.T with the 1/r_i scale folded into
                # the PSUM drain as a per-partition tensor_scalar
                rinv = half_state[c][2]
                st_i = c * tpc + t
                yp = ps_small.tile([128, D], F32, tag='small')
                if c % 2 == 1:
                    nc.tensor.matmul(yp[:, :],
                                     oT_lo[:, st_i * 128:(st_i + 1) * 128],
                                     woT2[0:VW, :], start=True, stop=True)
                else:
                    nc.tensor.matmul(yp[:, :],
                                     oT_hi[64:64 + VW,
                                           st_i * 128:(st_i + 1) * 128],
                                     woT2[64:64 + VW, :],
                                     start=True, stop=True)
                ysb = wkp.tile([128, D], F32, tag='ysb')
                if tail and t % 2:
                    # ScalarE is idle after the last exp: Copy-activation
                    # with a per-partition scale AP halves the tail's
                    # serial drain chain
                    nc.scalar.activation(ysb[:, :], yp[:, :],
                                         mybir.ActivationFunctionType.Copy,
                                         scale=rinv[:, t:t + 1])
                else:
                    nc.vector.tensor_scalar(ysb[:, :], yp[:, :],
                                            rinv[:, t:t + 1], None, MULT)
                # tail stores alternate queues so their ~0.6us issue costs
                # overlap instead of serializing on sync
                eng = nc.gpsimd if (tail and t % 2) else nc.sync
                eng.dma_start(out_d[st_i * 128:(st_i + 1) * 128, :],
                              ysb[:, :])

            # ---- projections preamble (DMA-paced) ----
            for c2 in range(n_chunks // 2):
                proj_qk2(c2)
                # derive qkB early: same q/k images, partition halves
                # swapped (chunk 0's STs gate on the first of these)
                c01 = slice(2 * c2 * CHUNK, (2 * c2 + 2) * CHUNK)
                nc.sync.dma_start(qkB[0:DH, c01], qkA[64:QKW, c01])
                nc.gpsimd.dma_start(qkB[64:QKW, c01], qkA[0:DH, c01])
                proj(vT[0:VW, :], wv_sb, 2 * c2, VW)
                proj(vT[0:VW, :], wv_sb, 2 * c2 + 1, VW)

            # ones row of v' (row 32; v dims 32-39 sit at rows 33-40 so
            # the r rows land on matmul-legal base partitions 32/96)
            nc.vector.memset(vT[32:33, :], 1.0)

            # ---- main loop over i-chunks ----
            # AV runs lag-16 (av(c) consumes chunk c's exp tiles during
            # chunk c+1, decoupled from the ACT stream) except av(6)/av(7),
            # which run lag-1 inside their own chunks so the tail only
            # handles chunk 7's half. Each chunk end drains one AV half;
            # each chunk c>=2 carries one half's post (oT copy, r column,
            # 4 y tiles); chunk 7 carries two halves' posts.
            pts_prev = None
            for c in range(n_chunks):
                pts = []
                cs = slice(c * CHUNK, (c + 1) * CHUNK)
                for g in range(gpc):
                    st = ps_st.tile([128, GJ * CHUNK], F32, tag='st2')
                    j0, j1 = GJ * g, GJ * g + 1
                    nc.tensor.matmul(
                        st[:, 0:CHUNK],
                        qkB[0:DH, j0 * 128:(j0 + 1) * 128], qkA[0:DH, cs],
                        start=True, stop=True)
                    nc.tensor.matmul(
                        st[:, CHUNK:2 * CHUNK],
                        qkA[64:QKW, j1 * 128:(j1 + 1) * 128],
                        qkB[64:QKW, cs],
                        start=True, stop=True)
                    pt = ptp.tile([128, GJ * CHUNK], BF16, tag='pt')
                    nc.scalar.activation(pt[:, :], st[:, :], EXP, scale=SCALE)
                    pts.append(pt)

                    if c == 0:
                        # v' transposes at 2/slot + junk keep-warm
                        transpose_v(2 * g)
                        transpose_v(2 * g + 1)
                        warm(1)
                    else:
                        if c < n_chunks - 1:
                            av_mm(c - 1, pts_prev[g], g)
                        if c >= n_chunks - 2 and g >= 1:
                            av_mm(c, pts[g - 1], g - 1)

                    if 2 <= c < n_chunks - 1:
                        if g == 0:
                            norm_b(c - 2)
                        elif g == 1:
                            norm_c(c - 2)
                        elif 2 <= g <= 5:
                            outproj_tile(c - 2, g - 2)
                    elif c == n_chunks - 1:
                        if g == 0:
                            norm_b(5)
                        elif g == 1:
                            norm_c(5)
                        elif 2 <= g <= 5:
                            outproj_tile(5, g - 2)
                        elif g == 6:
                            norm_b(6)
                        elif g == 7:
                            norm_c(6)
                        elif 8 <= g <= 11:
                            outproj_tile(6, g - 8)

                # chunk-end: finish lag-1 AV, drain completed halves
                if c == n_chunks - 2:
                    av_mm(c, pts[gpc - 1], gpc - 1)
                    norm_a(c - 1)
                    norm_a(c)
                elif 1 <= c < n_chunks - 2:
                    norm_a(c - 1)
                pts_prev = pts

            # ---- tail: chunk 7's half ----
            c = n_chunks - 1
            av_mm(c, pts_prev[gpc - 1], gpc - 1)
            norm_a(c, tail=True)
            norm_b(c)
            norm_c(c)
            for t in range(tpc):
                outproj_tile(c, t, tail=True)

    if split:
        _split_sync_waits(nc)
    return nc


def kernel(x, Wq, Wk, Wv, Wo, bo):
    x = np.asarray(x, dtype=np.float32)
    Wq = np.asarray(Wq, dtype=np.float32)
    Wk = np.asarray(Wk, dtype=np.float32)
    Wv = np.asarray(Wv, dtype=np.float32)
    Wo = np.asarray(Wo, dtype=np.float32)
    bo = np.asarray(bo, dtype=np.float32)

    if 'nc' not in _COMPILED:
        _COMPILED['nc'] = _build()
    nc = _COMPILED['nc']

    bf = ml_dtypes.bfloat16
    xT = np.ascontiguousarray(x.reshape(S, D).T).astype(bf)
    in_maps = []
    for h in range(N_CORES):
        sl = slice(h * DH, (h + 1) * DH)
        wqT = Wq[sl, :].T  # [320, 40]
        wkT = Wk[sl, :].T
        wvT = Wv[sl, :].T
        # prepacked images: per 128-row K chunk, q at cols +0..39 and k at
        # +64..103 (wA); v packed at 41-col stride with a zero col at 32
        # (the ones row of v' lives there, so r lands at partition 32/96)
        wA = np.zeros((128, 3 * QKW), dtype=np.float32)
        wv = np.zeros((128, 3 * VW), dtype=np.float32)
        for ci, kk in enumerate(KCH):
            o = sum(KCH[:ci])
            wA[0:kk, ci * QKW:ci * QKW + DH] = wqT[o:o + kk, :]
            wA[0:kk, ci * QKW + 64:ci * QKW + QKW] = wkT[o:o + kk, :]
            wv[0:kk, ci * VW:ci * VW + 32] = wvT[o:o + kk, 0:32]
            wv[0:kk, ci * VW + 33:ci * VW + VW] = wvT[o:o + kk, 32:40]
        woTh = Wo[:, sl].T  # [40, 320]
        woT = np.zeros((64 + VW, D), dtype=np.float32)
        for base in (0, 64):
            woT[base + 0:base + 32, :] = woTh[0:32, :]
            woT[base + 33:base + VW, :] = woTh[32:40, :]
        in_maps.append({
            'xT': xT,
            'wA': wA.astype(bf),
            'wv': wv.astype(bf),
            'woT': woT.astype(bf),
        })

    trace = bool(os.environ.get('BASS_KERNEL_TRACE'))

    def _run():
        return bass_utils.run_bass_kernel_spmd(
            nc, in_maps, core_ids=list(range(N_CORES)), trace=trace,
            tmpdir=os.environ.get('BASS_KERNEL_TRACE_DIR') or None)

    try:
        res = _run()
    except Exception:
        # A previously crashed NEFF can leave the device unrecoverable; the
        # failed attempt clears it, so one retry is usually enough.
        res = _run()
    _COMPILED['last_res'] = res

    acc = res.results[0]['out'].astype(np.float32).copy()
    for h in range(1, N_CORES):
        acc += res.results[h]['out']
    acc += bo[None, :]
    return acc.reshape(1, S, D)


# revision 65
# speedup vs baseline: 1.0647x; 1.0647x over previous
"""CrossAttention (B=1, S=4096, H=8, DH=40) on 8 Trainium2 NeuronCores.

Sharding: tensor-parallel over the 8 heads — core h computes head h's full
attention plus its partial output projection; the host sums the 8 partials
and adds the bias.

v12 dataflow (uniform lag-1):
  warm-up   dense junk matmuls first: the PE clock release (1.2 -> 2.4 GHz)
            follows ~10us of gap-free activity, so warming starts before
            any real work; a dummy ACT preloads the exp table meanwhile
  preamble  prepacked weight images (single DMAs); xT in 1024-col slices
            over two issuing queues; qkA projections drain from idle ps_st
            banks (low chunk on DVE, high on ScalarE); qkB derived from
            qkA by two SBUF-SBUF DMAs (same data, partition halves
            swapped); v-projection and v' PE-transposes also live here
  ST        [128j, 512i] pairs on disjoint PE row groups -> exp on ScalarE
            [128, 1024] (the wall: ~16.7M exps at 1 elem/cycle/lane)
  AV        lag-1 everywhere: av(c) consumes chunk c's exp tiles one slot
            behind; pairs of chunks share one PSUM bank (even chunk at PE
            cols 0-40, odd at 64-104 via tile_position), pair banks
            alternate between ps_av and ps_warm
  norm      no broadcast, no row reciprocal: y_i = (o_i @ Wo) / r_i -- the
            r row is PE-transposed into a per-partition column, 1/r is a
            cheap [128,4] DVE reciprocal, and the divide folds into the
            y-tile PSUM drain as a per-partition tensor_scalar
  post      each chunk's half-pair post-processing (oT copy, r column,
            4 y tiles) runs early in the NEXT chunk; the tail handles only
            chunk 7's half
"""

import os

import ml_dtypes
import numpy as np

import concourse.bass as bass
import concourse.mybir as mybir
from concourse import bass_utils, masks
from concourse.tile import TileContext

S = 4096
D = 320
H = 8
DH = 40
N_CORES = 8
CHUNK = 512               # i-chunk width (one fp32 PSUM bank)
VW = 41                   # v' stationary width: 40 v cols, ones col 40
GJ = 2                    # j-tiles per exp group (2 PSUM banks)
SCALE = float(DH) ** -0.5
QKW = 104
BURST = int(os.environ.get('BASS_BURST', '16'))  # warm-up burst length

F32 = mybir.dt.float32
BF16 = mybir.dt.bfloat16
EXP = mybir.ActivationFunctionType.Exp
MULT = mybir.AluOpType.mult

KCH = (128, 128, 64)      # K chunks of D=320

_COMPILED = {}


def _split_sync_waits(nc, max_waits=1):
    """This walrus build rejects instructions with more than one sync wait.
    Spill the excess onto same-engine nops placed just before the
    instruction (engine streams execute in program order, so all waits are
    satisfied before the instruction issues)."""
    for f in nc.m.functions:
        for bb in f.blocks:
            out = []
            changed = False
            for inst in bb.instructions:
                si = inst.sync_info
                if si is not None and si.on_wait and len(si.on_wait) > max_waits:
                    waits = list(si.on_wait)
                    for i in range(max_waits, len(waits), max_waits):
                        nop = mybir.InstNoOp(
                            name=nc.get_next_instruction_name(),
                            engine=inst.engine,
                            bass_nofuse=True,
                            sync_info=mybir.SyncInfo(
                                on_wait=waits[i:i + max_waits], on_update=[]),
                        )
                        out.append(nop)
                    inst.sync_info = mybir.SyncInfo(
                        on_wait=waits[:max_waits],
                        on_update=list(si.on_update or []))
                    changed = True
                out.append(inst)
            if changed:
                bb.instructions = out


def _build(s=None, split=True):
    s = s or S
    n_chunks = s // CHUNK
    jt = s // 128
    gpc = jt // GJ            # exp groups per chunk
    tpc = CHUNK // 128        # s-tiles per chunk
    nc = bass.Bass('TRN2', target_bir_lowering=False, debug=False)

    xT_d = nc.dram_tensor('xT', [D, s], BF16, kind='ExternalInput').ap()
    wA_d = nc.dram_tensor('wA', [128, 3 * QKW], BF16, kind='ExternalInput').ap()
    wv_d = nc.dram_tensor('wv', [128, 3 * VW], BF16, kind='ExternalInput').ap()
    woT_d = nc.dram_tensor('woT', [64 + VW, D], BF16, kind='ExternalInput').ap()
    out_d = nc.dram_tensor('out', [s, D], F32, kind='ExternalOutput').ap()

    with TileContext(nc) as tc:
        with tc.tile_pool(name='const', bufs=1) as cpool, \
             tc.tile_pool(name='big', bufs=1) as big, \
             tc.tile_pool(name='pt', bufs=18) as ptp, \
             tc.tile_pool(name='work', bufs=3) as wkp, \
             tc.tile_pool(name='ps_st', bufs=2, space='PSUM') as ps_st, \
             tc.tile_pool(name='ps_small', bufs=2, space='PSUM') as ps_small, \
             tc.tile_pool(name='ps_av', bufs=1, space='PSUM') as ps_av, \
             tc.tile_pool(name='ps_warm', bufs=1, space='PSUM') as ps_warm:

            # ---- warm-up burst (first thing emitted) ----
            jsrc = cpool.tile([128, CHUNK], BF16, tag='jsrc')
            nc.vector.memset(jsrc[:, :], 0.25)
            for _ in range(BURST):
                wj = ps_warm.tile([128, CHUNK], F32, tag='av', name='wj')
                nc.tensor.matmul(wj[:, :], jsrc[:, 0:128], jsrc[:, :],
                                 start=True, stop=True)
            # dummy ACT: exp table-set load (~2.7us) during the DMA wait
            sc = wkp.tile([1, 16], F32, tag='sc')
            nc.scalar.activation(sc[0:1, :], jsrc[0:1, 0:16], EXP)

            # ---- constants & inputs ----
            ident = cpool.tile([128, 128], F32, tag='ident')
            masks.make_identity(nc, ident[:, :])

            wA = cpool.tile([128, 3 * QKW], BF16, tag='wA')
            wv_sb = cpool.tile([128, 3 * VW], BF16, tag='wv')
            woT2 = cpool.tile([64 + VW, D], BF16, tag='woT')
            nc.sync.dma_start(wA[:, :], wA_d)
            nc.scalar.dma_start(wv_sb[:, :], wv_d)
            nc.gpsimd.dma_start(woT2[:, :], woT_d)

            # xT in 1024-col slices; scalar queue stays clear of slice DMAs
            # so the table load + qkA drains there are never queued behind
            xt0 = big.tile([128, s], BF16, tag='xt0')
            xt1 = big.tile([128, s], BF16, tag='xt1')
            xt2 = big.tile([64, s], BF16, tag='xt2')
            xts = (xt0, xt1, xt2)
            for sl in range(s // 1024):
                ss = slice(sl * 1024, (sl + 1) * 1024)
                nc.sync.dma_start(xt0[:, ss], xT_d[0:128, ss])
                nc.sync.dma_start(xt1[:, ss], xT_d[128:256, ss])
                nc.gpsimd.dma_start(xt2[:, ss], xT_d[256:320, ss])

            qkA = big.tile([QKW, s], BF16, tag='qkA')
            qkB = big.tile([QKW, s], BF16, tag='qkB')
            vT = big.tile([VW, s], F32, tag='vT')
            vsb = big.tile([128, jt * VW], BF16, tag='vsb')
            # ones column of v' at block col 32 (so the AV r rows land on
            # matmul-legal base partitions 32/96)
            for j in range(jt):
                nc.vector.memset(vsb[:, j * VW + 32:j * VW + 33], 1.0)
            oT_lo = big.tile([VW, s], BF16, tag='oT_lo')
            oT_hi = big.tile([64 + VW, s], BF16, tag='oT_hi')


            # ---- helpers ----
            def proj(dst, w_sb, c, ww):
                ps = ps_small.tile([QKW, CHUNK], F32, tag='small')
                for ci, kk in enumerate(KCH):
                    nc.tensor.matmul(
                        ps[0:ww, :],
                        w_sb[0:kk, ci * ww:(ci + 1) * ww],
                        xts[ci][0:kk, c * CHUNK:(c + 1) * CHUNK],
                        start=(ci == 0), stop=(ci == 2))
                nc.vector.tensor_copy(dst[:, c * CHUNK:(c + 1) * CHUNK],
                                      ps[0:ww, :])

            def proj_qk2(c2):
                # qkA projections for chunks 2*c2, 2*c2+1 into one 2-bank
                # ps_st tile (idle until the ST stream starts); 4-chunk WAR
                # distance keeps the PE clear of the drain round-trip
                ps = ps_st.tile([128, GJ * CHUNK], F32, tag='st2')
                for h in range(2):
                    c = 2 * c2 + h
                    cs = slice(c * CHUNK, (c + 1) * CHUNK)
                    for ci, kk in enumerate(KCH):
                        nc.tensor.matmul(
                            ps[0:QKW, h * CHUNK:(h + 1) * CHUNK],
                            wA[0:kk, ci * QKW:(ci + 1) * QKW],
                            xts[ci][0:kk, cs],
                            start=(ci == 0), stop=(ci == 2))
                c0s = slice(2 * c2 * CHUNK, (2 * c2 + 1) * CHUNK)
                c1s = slice((2 * c2 + 1) * CHUNK, (2 * c2 + 2) * CHUNK)
                nc.vector.tensor_copy(qkA[:, c0s], ps[0:QKW, 0:CHUNK])
                nc.scalar.copy(qkA[:, c1s], ps[0:QKW, CHUNK:2 * CHUNK])

            def transpose_v(j):
                tp = ps_small.tile([128, VW], F32, tag='small')
                nc.tensor.transpose(tp[:, 0:VW], vT[:, j * 128:(j + 1) * 128],
                                    ident[0:VW, 0:VW])
                # skip col 32 -- it holds the pre-set ones column
                nc.vector.tensor_copy(vsb[:, j * VW:j * VW + 32],
                                      tp[:, 0:32])
                nc.vector.tensor_copy(vsb[:, j * VW + 33:(j + 1) * VW],
                                      tp[:, 33:VW])

            def warm(n=1):
                # junk keep-warm matmuls for thin chunk-0 slots
                for _ in range(n):
                    wj = ps_warm.tile([128, CHUNK], F32, tag='av',
                                      name='wj')
                    nc.tensor.matmul(wj[0:QKW, 0:384], wA[:, 0:QKW],
                                     xt0[:, 0:384], start=True, stop=True)

            av_tiles = {}

            def av_mm(c_src, pt, g):
                # two j-tile matmuls of group g, consuming chunk c_src's exp
                # tiles: even chunk at PE cols 0-40, odd at 64-104. Pair
                # banks alternate ps_av / ps_warm so a new pair never WARs
                # on the previous pair's drain copy.
                lo = (c_src % 2 == 1)
                pair = c_src // 2
                for k in range(GJ):
                    j = GJ * g + k
                    if j == 0 and c_src % 2 == 0:
                        pool = ps_av if pair % 2 == 0 else ps_warm
                        av_tiles[pair] = pool.tile(
                            [128, CHUNK], F32, tag='av', name='av')
                    av = av_tiles[pair]
                    if lo:
                        nc.tensor.matmul(
                            av[0:VW, :], vsb[:, j * VW:(j + 1) * VW],
                            pt[:, k * CHUNK:(k + 1) * CHUNK],
                            start=(j == 0), stop=(j == jt - 1))
                    else:
                        nc.tensor.matmul(
                            av[64:64 + VW, :], vsb[:, j * VW:(j + 1) * VW],
                            pt[:, k * CHUNK:(k + 1) * CHUNK],
                            start=(j == 0), stop=(j == jt - 1),
                            tile_position=(0, 64))

            half_state = {}

            def norm_a(c, tail=False):
                # drain this chunk's 41 AV rows (r row included at
                # partition 40 or 104 -- transposed in place by norm_c, no
                # partition-0 DMA hop needed)
                av = av_tiles[c // 2]
                row0 = 0 if c % 2 == 1 else 64
                m = wkp.tile([105, CHUNK], F32, tag='m')
                nc.vector.tensor_copy(m[row0:row0 + VW, :],
                                      av[row0:row0 + VW, :])
                half_state[c] = [m, None, None]

            def norm_b(c):
                # unnormalized oT image (bf16 copy; SBUF 2x mode)
                m = half_state[c][0]
                cs = slice(c * CHUNK, (c + 1) * CHUNK)
                if c % 2 == 1:
                    nc.vector.tensor_copy(oT_lo[:, cs], m[0:VW, :])
                else:
                    nc.vector.tensor_copy(oT_hi[64:64 + VW, cs],
                                          m[64:64 + VW, :])

            def norm_c(c):
                # PE-transpose the r row (read in place at partition 40 or
                # 104 -- SBUF partition k feeds PE array row k, so the
                # matching identity diagonal element makes the transpose
                # land in rr[:, t]) and take the cheap [128,4] reciprocal
                m = half_state[c][0]
                rr = ps_small.tile([128, 8], F32, tag='small')
                if c % 2 == 1:
                    src_r, rp = m, 32
                else:
                    src_r = wkp.tile([1, CHUNK], F32, tag='r')
                    nc.gpsimd.dma_start(src_r[0:1, :], m[96:97, :])
                    rp = 0
                for t in range(tpc):
                    nc.tensor.transpose(
                        rr[:, t:t + 1],
                        src_r[rp:rp + 1, t * 128:(t + 1) * 128],
                        ident[rp:rp + 1, rp:rp + 1])
                rcol = wkp.tile([128, 8], F32, tag='rcol')
                nc.vector.tensor_copy(rcol[:, 0:tpc], rr[:, 0:tpc])
                rinv = wkp.tile([128, 8], F32, tag='rinv')
                nc.vector.reciprocal(rinv[:, 0:tpc], rcol[:, 0:tpc])
                half_state[c][2] = rinv

            def outproj_tile(c, t, tail=False):
                # y tile = oT.T @ Wo_h.T with the 1/r_i scale folded into
                # the PSUM drain as a per-partition tensor_scalar
                rinv = half_state[c][2]
                st_i = c * tpc + t
                yp = ps_small.tile([128, D], F32, tag='small')
                if c % 2 == 1:
                    nc.tensor.matmul(yp[:, :],
                                     oT_lo[:, st_i * 128:(st_i + 1) * 128],
                                     woT2[0:VW, :], start=True, stop=True)
                else:
                    nc.tensor.matmul(yp[:, :],
                                     oT_hi[64:64 + VW,
                                           st_i * 128:(st_i + 1) * 128],
                                     woT2[64:64 + VW, :],
                                     start=True, stop=True)
                ysb = wkp.tile([128, D], F32, tag='ysb')
                if tail and t % 2:
                    # ScalarE is idle after the last exp: Copy-activation
                    # with a per-partition scale AP halves the tail's
                    # serial drain chain
                    nc.scalar.activation(ysb[:, :], yp[:, :],
                                         mybir.ActivationFunctionType.Copy,
                                         scale=rinv[:, t:t + 1])
                else:
                    nc.vector.tensor_scalar(ysb[:, :], yp[:, :],
                                            rinv[:, t:t + 1], None, MULT)
                # tail stores alternate queues so their ~0.6us issue costs
                # overlap instead of serializing on sync
                eng = nc.gpsimd if (tail and t % 2) else nc.sync
                eng.dma_start(out_d[st_i * 128:(st_i + 1) * 128, :],
                              ysb[:, :])

            # ---- projections preamble (DMA-paced) ----
            for c2 in range(n_chunks // 2):
                proj_qk2(c2)
                # derive qkB early: same q/k images, partition halves
                # swapped (chunk 0's STs gate on the first of these)
                c01 = slice(2 * c2 * CHUNK, (2 * c2 + 2) * CHUNK)
                nc.sync.dma_start(qkB[0:DH, c01], qkA[64:QKW, c01])
                nc.gpsimd.dma_start(qkB[64:QKW, c01], qkA[0:DH, c01])
                proj(vT[0:VW, :], wv_sb, 2 * c2, VW)
                proj(vT[0:VW, :], wv_sb, 2 * c2 + 1, VW)

            # ---- main loop over i-chunks ----
            # AV runs lag-16 (av(c) consumes chunk c's exp tiles during
            # chunk c+1, decoupled from the ACT stream) except av(6)/av(7),
            # which run lag-1 inside their own chunks so the tail only
            # handles chunk 7's half. Each chunk end drains one AV half;
            # each chunk c>=2 carries one half's post (oT copy, r column,
            # 4 y tiles); chunk 7 carries two halves' posts.
            pts_prev = None
            for c in range(n_chunks):
                pts = []
                cs = slice(c * CHUNK, (c + 1) * CHUNK)
                for g in range(gpc):
                    st = ps_st.tile([128, GJ * CHUNK], F32, tag='st2')
                    j0, j1 = GJ * g, GJ * g + 1
                    nc.tensor.matmul(
                        st[:, 0:CHUNK],
                        qkB[0:DH, j0 * 128:(j0 + 1) * 128], qkA[0:DH, cs],
                        start=True, stop=True)
                    nc.tensor.matmul(
                        st[:, CHUNK:2 * CHUNK],
                        qkA[64:QKW, j1 * 128:(j1 + 1) * 128],
                        qkB[64:QKW, cs],
                        start=True, stop=True)
                    pt = ptp.tile([128, GJ * CHUNK], BF16, tag='pt')
                    nc.scalar.activation(pt[:, :], st[:, :], EXP, scale=SCALE)
                    pts.append(pt)

                    if c == 0:
                        # v' transposes at 2/slot + junk keep-warm
                        transpose_v(2 * g)
                        transpose_v(2 * g + 1)
                        warm(1)
                    else:
                        if c < n_chunks - 1:
                            av_mm(c - 1, pts_prev[g], g)
                        if c >= n_chunks - 2 and g >= 1:
                            av_mm(c, pts[g - 1], g - 1)

                    if 2 <= c < n_chunks - 1:
                        if g == 0:
                            norm_b(c - 2)
                        elif g == 1:
                            norm_c(c - 2)
                        elif 2 <= g <= 5:
                            outproj_tile(c - 2, g - 2)
                    elif c == n_chunks - 1:
                        if g == 0:
                            norm_b(5)
                        elif g == 1:
                            norm_c(5)
                        elif 2 <= g <= 5:
                            outproj_tile(5, g - 2)
                        elif g == 6:
                            norm_b(6)
                        elif g == 7:
                            norm_c(6)
                        elif 8 <= g <= 11:
                            outproj_tile(6, g - 8)

                # chunk-end: finish lag-1 AV, drain completed halves
                if c == n_chunks - 2:
                    av_mm(c, pts[gpc - 1], gpc - 1)
                    norm_a(c - 1)
                    norm_a(c)
                elif 1 <= c < n_chunks - 2:
                    norm_a(c - 1)
                pts_prev = pts

            # ---- tail: chunk 7's half ----
            c = n_chunks - 1
            av_mm(c, pts_prev[gpc - 1], gpc - 1)
            norm_a(c, tail=True)
            norm_b(c)
            norm_c(c)
            for t in range(tpc):
                outproj_tile(c, t, tail=True)

    if split:
        _split_sync_waits(nc)
    return nc


def kernel(x, Wq, Wk, Wv, Wo, bo):
    x = np.asarray(x, dtype=np.float32)
    Wq = np.asarray(Wq, dtype=np.float32)
    Wk = np.asarray(Wk, dtype=np.float32)
    Wv = np.asarray(Wv, dtype=np.float32)
    Wo = np.asarray(Wo, dtype=np.float32)
    bo = np.asarray(bo, dtype=np.float32)

    if 'nc' not in _COMPILED:
        _COMPILED['nc'] = _build()
    nc = _COMPILED['nc']

    bf = ml_dtypes.bfloat16
    xT = np.ascontiguousarray(x.reshape(S, D).T).astype(bf)
    in_maps = []
    for h in range(N_CORES):
        sl = slice(h * DH, (h + 1) * DH)
        wqT = Wq[sl, :].T  # [320, 40]
        wkT = Wk[sl, :].T
        wvT = Wv[sl, :].T
        # prepacked images: per 128-row K chunk, q at cols +0..39 and k at
        # +64..103 (wA); v packed at 41-col stride with a zero col at 32
        # (the ones row of v' lives there, so r lands at partition 32/96)
        wA = np.zeros((128, 3 * QKW), dtype=np.float32)
        wv = np.zeros((128, 3 * VW), dtype=np.float32)
        for ci, kk in enumerate(KCH):
            o = sum(KCH[:ci])
            wA[0:kk, ci * QKW:ci * QKW + DH] = wqT[o:o + kk, :]
            wA[0:kk, ci * QKW + 64:ci * QKW + QKW] = wkT[o:o + kk, :]
            wv[0:kk, ci * VW:ci * VW + 32] = wvT[o:o + kk, 0:32]
            wv[0:kk, ci * VW + 33:ci * VW + VW] = wvT[o:o + kk, 32:40]
        woTh = Wo[:, sl].T  # [40, 320]
        woT = np.zeros((64 + VW, D), dtype=np.float32)
        for base in (0, 64):
            woT[base + 0:base + 32, :] = woTh[0:32, :]
            woT[base + 33:base + VW, :] = woTh[32:40, :]
        in_maps.append({
            'xT': xT,
            'wA': wA.astype(bf),
            'wv': wv.astype(bf),
            'woT': woT.astype(bf),
        })

    trace = bool(os.environ.get('BASS_KERNEL_TRACE'))

    def _run():
        return bass_utils.run_bass_kernel_spmd(
            nc, in_maps, core_ids=list(range(N_CORES)), trace=trace,
            tmpdir=os.environ.get('BASS_KERNEL_TRACE_DIR') or None)

    try:
        res = _run()
    except Exception:
        # A previously crashed NEFF can leave the device unrecoverable; the
        # failed attempt clears it, so one retry is usually enough.
        res = _run()
    _COMPILED['last_res'] = res

    acc = res.results[0]['out'].astype(np.float32).copy()
    for h in range(1, N_CORES):
        acc += res.results[h]['out']
    acc += bo[None, :]
    return acc.reshape(1, S, D)
